# revision 27
# baseline (speedup 1.0000x reference)
"""Trainium2 Bass kernel for batched CRF negative log-likelihood.

Meet-in-the-middle forward algorithm (device), probability space:
  forward chain:  a_{t+1} = (Wf @ a_t) * EF_t,  a_0 = onehot(START)
  backward chain: v_{k+1} = (Wb @ v_k) * EB_k,  v_0 = Wstop * E_{L-1}
     where Wb = W^T per 25-state group; EB_k = E_{L-2-k}, except at a
     sequence's final backward step where EB = 1 (so v_K = beta_m pure).
  Z = sum_j a_m[j] * beta_m[j],  m = ceil(L/2), K = floor(L/2).
Both chains are independent 256-tick scans (vs 513 for pure forward),
halving the sequential-latency-bound wall time.

Layout: 4 groups of 25 states on partitions (104 rows incl 4 renorm
rows); 64 columns per group = 256 seqs/core, sorted by length (desc)
and dealt round-robin over 8 cores.  Columns deactivate as chains end
(compile-time schedule).  Every W steps the state is rescaled by
m = 1/r (r = stop-projection for F, group-sum for B) folded into the E
tile; m is dumped so the host can undo it exactly.  The full state ring
is dumped to DRAM every DUMPG ticks; the host reads a_m / beta_m at each
sequence's own meet tick, computes log(a_m . beta_m) + scale corrections
+ cumulative feat-max, subtracts the gold path score, and averages.
"""

import sys

sys.path.insert(0, "/opt/trn_rl_repo")

import numpy as np
import ml_dtypes

bf16 = ml_dtypes.bfloat16

# ---- problem constants (hardcoded per contest rules) ----
B, T, OUT = 2048, 512, 23
K = OUT + 2
START, STOP = OUT, OUT + 1
NEG = -10000.0

NCORES = 8
G = 4            # state groups on partitions (4 x 25 = 100 state rows)
NMAX = 64        # max columns = (2048/8)/G
RING = 32        # p ring depth (steps)
W = 16           # renormalization period (steps)
LAG = 4          # staleness of r used for renormalization (= prep lead time)
CH = 32          # E-chunk size in steps
DUMPG = 16       # state-dump group size (ring slots per dump DMA)
SEQ_PER_CORE = B // NCORES


# ----------------------------------------------------------------------------
# schedule (compile-time, from lengths)
# ----------------------------------------------------------------------------
def make_schedule(lengths, renorm_w=W):
    lengths = np.asarray(lengths).astype(np.int64)
    order = np.argsort(-lengths, kind="stable")
    mhalf = -(-lengths // 2)                  # m_s = ceil(L/2): forward ticks
    khalf = lengths // 2                      # K_s = floor(L/2): backward ticks
    TB = int(max(mhalf.max(), khalf.max()))   # device steps 0..TB-1

    def widths(active_ticks):
        # N_t = cols processed at step t; col for seq alive while t < ticks
        Ag = np.array([(active_ticks > t).sum() for t in range(TB)],
                      dtype=np.int64)
        Acore = -(-Ag // NCORES)
        N_t = np.maximum(1, -(-Acore // G)).astype(int)
        off = np.zeros(TB + 1, dtype=np.int64)
        for t in range(TB):
            off[t + 1] = off[t] + N_t[t]
        return N_t, off, int(off[TB])

    NF_t, offF, ECF = widths(mhalf)
    NB_t, offB, ECB = widths(khalf)
    # device renorm steps; empty when the host pre-folds scales into E
    applies = list(range(renorm_w, TB, renorm_w)) if renorm_w else []
    return dict(order=order, mhalf=mhalf, khalf=khalf, TB=TB,
                NF_t=NF_t, offF=offF, ECF=ECF,
                NB_t=NB_t, offB=offB, ECB=ECB, applies=applies)


# ----------------------------------------------------------------------------
# host-side input preparation (per core)
# ----------------------------------------------------------------------------
def pos(g, j):
    """Partition of state j of group g.  r-rows live at 96..99 (32-aligned
    for the dump DMA / rcp reads); group 3's states fill 75..95 + 100..103."""
    if g < 3:
        return 25 * g + j
    return 75 + j if j < 21 else 100 + (j - 21)


def rpos(g):
    return 96 + g


ROWMAP = np.array([[pos(g, j) for j in range(K)] for g in range(G)])


def build_walls(transitions):
    M = np.exp(transitions.astype(np.float64)).astype(np.float32)      # [K, K]
    Mstop = np.exp(transitions[STOP].astype(np.float64)).astype(np.float32)
    Wf = np.zeros((104, 104), dtype=np.float32)   # [out_row, in_row]
    Wb = np.zeros((104, 104), dtype=np.float32)
    for g in range(G):
        for jo in range(K):
            for ji in range(K):
                Wf[pos(g, jo), pos(g, ji)] = M[jo, ji]
                Wb[pos(g, jo), pos(g, ji)] = M[ji, jo]   # transposed block
        for ji in range(K):
            Wf[rpos(g), pos(g, ji)] = Mstop[ji]          # stop projection
            Wb[rpos(g), pos(g, ji)] = 1.0                # group mass
    lhsTf = np.ascontiguousarray(Wf.T).astype(bf16)      # [in(contract), out]
    lhsTb = np.ascontiguousarray(Wb.T).astype(bf16)
    return lhsTf, lhsTb, Mstop


def build_p0():
    p0 = np.zeros((104, NMAX), dtype=np.float32)
    for g in range(G):
        p0[pos(g, START), :] = 1.0
    return p0.astype(bf16)


def build_wones():
    """lhsT for the m-broadcast matmul: out[:, c] = ones_block @ m[:, c].
    All of group g's state rows and its r-row get m[g]."""
    w = np.zeros((4, 104), dtype=np.float32)
    for g in range(G):
        for j in range(K):
            w[g, pos(g, j)] = 1.0
        w[g, rpos(g)] = 1.0
    return w


def build_core_inputs(feats_shard, lens_s, sched, Mstop, c0=None):
    """feats_shard: [256, T, K] f32 for this core (order-sorted slice).
    Returns dict of device input arrays + mu [256, T]."""
    m_s = -(-lens_s // 2)
    k_s = lens_s // 2
    NF_t, offF, ECF = sched["NF_t"], sched["offF"], sched["ECF"]
    NB_t, offB, ECB = sched["NB_t"], sched["offB"], sched["ECB"]
    TB = sched["TB"]

    mu = feats_shard.max(-1)                                   # [256, T]
    Eraw = np.exp(feats_shard - mu[..., None])                 # [256, T, K]
    if c0 is not None:
        # host-side renorm: fold the estimated per-tick log-growth into E
        # (exactly like mu); undone via mu_cum in the assembly.
        inc = np.clip(np.log(Eraw.sum(-1)) + c0, -30.0, 30.0)  # [256, T]
        Eraw = Eraw * np.exp(-inc)[..., None]
        mu = mu + inc
    E = Eraw.astype(bf16)                                      # [256, T, K]
    # seq s = n*G + g  ->  row pos(g, j), col n
    Er = E.reshape(NMAX, G, T, K).transpose(1, 3, 2, 0)        # [G, K, T, NMAX]

    efF = np.ones((104, ECF), dtype=bf16)
    for t in range(TB):
        n = NF_t[t]
        for g in range(G):
            efF[ROWMAP[g], offF[t]:offF[t] + n] = Er[g, :, t, :n]

    # backward: step t uses E_{L-2-t} for each seq, ones at final step t=K-1
    efB = np.ones((104, ECB), dtype=bf16)
    tidx = np.asarray(lens_s) - 2 - np.arange(TB)[:, None]     # [TB, 256]
    for t in range(TB):
        n = NB_t[t]
        for g in range(G):
            for nn in range(n):
                s = nn * G + g
                if t >= k_s[s]:          # dead (rounding slack): leave ones
                    continue
                if t == k_s[s] - 1:      # final backward step: no emission
                    continue
                efB[ROWMAP[g], offB[t] + nn] = Er[g, :, tidx[t, s], nn]

    # v_0 = Wstop * E_{L-1} per seq
    p0B = np.ones((104, NMAX), dtype=np.float32)
    EL = E[np.arange(SEQ_PER_CORE), np.asarray(lens_s) - 1].astype(np.float32)
    for g in range(G):
        for nn in range(NMAX):
            s = nn * G + g
            p0B[ROWMAP[g], nn] = Mstop * EL[s]
    return dict(efF=efF, efB=efB, p0B=p0B.astype(bf16)), mu


# ----------------------------------------------------------------------------
# device kernel builder
# ----------------------------------------------------------------------------
def build_nc(sched, repeat=1, eng_f="vector", eng_b="vector", warm=0,
             warm_w=48, do_dump=True, do_renorm=True, emul="stt", pbufs=2):
    import concourse.bass as bass
    import concourse.tile as tile
    from concourse import bacc, mybir

    TB, applies = sched["TB"], sched["applies"]
    NAPPLY = len(applies)
    NDUMP = -(-(TB + 1) // DUMPG)

    nc = bacc.Bacc("TRN2", target_bir_lowering=False, debug=False,
                   num_devices=NCORES)
    efF = nc.dram_tensor("efF", [104, sched["ECF"]], mybir.dt.bfloat16,
                         kind="ExternalInput").ap()
    efB = nc.dram_tensor("efB", [104, sched["ECB"]], mybir.dt.bfloat16,
                         kind="ExternalInput").ap()
    p0F = nc.dram_tensor("p0F", [104, NMAX], mybir.dt.bfloat16,
                         kind="ExternalInput").ap()
    p0B = nc.dram_tensor("p0B", [104, NMAX], mybir.dt.bfloat16,
                         kind="ExternalInput").ap()
    wallF = nc.dram_tensor("wallF", [104, 104], mybir.dt.bfloat16,
                           kind="ExternalInput").ap()
    wallB = nc.dram_tensor("wallB", [104, 104], mybir.dt.bfloat16,
                           kind="ExternalInput").ap()
    wones = nc.dram_tensor("wones", [4, 104], mybir.dt.float32,
                           kind="ExternalInput").ap()
    dumpF = nc.dram_tensor("dumpF", [104, NDUMP * DUMPG * NMAX],
                           mybir.dt.bfloat16, kind="ExternalOutput").ap()
    dumpB = nc.dram_tensor("dumpB", [104, NDUMP * DUMPG * NMAX],
                           mybir.dt.bfloat16, kind="ExternalOutput").ap()
    mdF = nc.dram_tensor("mdF", [4, max(1, NAPPLY) * NMAX], mybir.dt.float32,
                         kind="ExternalOutput").ap()
    mdB = nc.dram_tensor("mdB", [4, max(1, NAPPLY) * NMAX], mybir.dt.float32,
                         kind="ExternalOutput").ap()

    with tile.TileContext(nc) as tc:
        from contextlib import ExitStack
        with ExitStack() as ctx:
            singles = ctx.enter_context(tc.tile_pool(name="singles", bufs=1))
            epoolF = ctx.enter_context(tc.tile_pool(name="epoolF", bufs=3))
            epoolB = ctx.enter_context(tc.tile_pool(name="epoolB", bufs=3))
            psumF = ctx.enter_context(
                tc.tile_pool(name="psumF", bufs=pbufs, space="PSUM"))
            psumB = ctx.enter_context(
                tc.tile_pool(name="psumB", bufs=pbufs, space="PSUM"))
            mbcpool = ctx.enter_context(
                tc.tile_pool(name="mbcpool", bufs=2, space="PSUM"))
            efoldpool = ctx.enter_context(tc.tile_pool(name="efoldpool",
                                                       bufs=4))
            dummypool = (ctx.enter_context(
                tc.tile_pool(name="dummypool", bufs=2, space="PSUM"))
                if warm else None)

            wallF_t = singles.tile([104, 104], mybir.dt.bfloat16)
            nc.sync.dma_start(out=wallF_t[:], in_=wallF[:])
            wallB_t = singles.tile([104, 104], mybir.dt.bfloat16)
            nc.sync.dma_start(out=wallB_t[:], in_=wallB[:])
            wones_t = singles.tile([4, 104], mybir.dt.float32)
            nc.sync.dma_start(out=wones_t[:], in_=wones[:])
            ringF = singles.tile([104, RING * NMAX], mybir.dt.bfloat16)
            nc.vector.memset(ringF[:, NMAX:], 0.0)
            nc.sync.dma_start(out=ringF[:, 0:NMAX], in_=p0F[:])
            ringB = singles.tile([104, RING * NMAX], mybir.dt.bfloat16)
            nc.vector.memset(ringB[:, NMAX:], 0.0)
            nc.sync.dma_start(out=ringB[:, 0:NMAX], in_=p0B[:])
            mringF = singles.tile([4, max(1, NAPPLY) * NMAX], mybir.dt.float32)
            nc.vector.memset(mringF[:], 1.0)
            mringB = singles.tile([4, max(1, NAPPLY) * NMAX], mybir.dt.float32)
            nc.vector.memset(mringB[:], 1.0)

            nchunks = -(-TB // CH)

            engs = {"vector": nc.vector, "gpsimd": nc.gpsimd}
            chains = {}
            for key, epool, psum, mring, ring, wt, ef, dump, N_t, off, ve in (
                ("F", epoolF, psumF, mringF, ringF, wallF_t, efF, dumpF,
                 sched["NF_t"], sched["offF"], engs[eng_f]),
                ("B", epoolB, psumB, mringB, ringB, wallB_t, efB, dumpB,
                 sched["NB_t"], sched["offB"], engs[eng_b])):
                chains[key] = dict(epool=epool, psum=psum, mring=mring,
                                   ring=ring, wt=wt, ef=ef, dump=dump,
                                   N_t=N_t, off=off, ve=ve,
                                   chunk_w=[int(off[min((c + 1) * CH, TB)]
                                                - off[c * CH])
                                            for c in range(nchunks)],
                                   echunks=[None] * nchunks)
            for ch in chains.values():
                ch["maxw"] = max(ch["chunk_w"])

            def load_chunk(ch, c):
                wdt = ch["chunk_w"][c]
                et = ch["epool"].tile([104, ch["maxw"]], mybir.dt.bfloat16,
                                      tag="E")
                a = int(ch["off"][c * CH])
                nc.sync.dma_start(out=et[:, 0:wdt], in_=ch["ef"][:, a:a + wdt])
                ch["echunks"][c] = et

            # renorm pipeline, spread so no engine stalls another:
            #   tick ta-LAG:   reciprocal of r rows (DVE, after stts)
            #   tick ta-LAG+1: broadcast matmul (PE, after chain matmuls)
            #   tick ta-LAG+2: fold multiply into E copy (DVE)
            #   tick ta:       stt consumes folded E
            rcp_at = {t - LAG: t for t in applies}
            mbc_at = {t - LAG + 1: t for t in applies}
            mul_at = {t - LAG + 2: t for t in applies}

            def body(_i=None):
              if _i is not None:
                nc.sync.dma_start(out=ringF[:, 0:NMAX], in_=p0F[:])
                nc.sync.dma_start(out=ringB[:, 0:NMAX], in_=p0B[:])
              for ch in chains.values():
                for c_ in range(nchunks):
                    ch["echunks"][c_] = None
                load_chunk(ch, 0)
                if nchunks > 1:
                    load_chunk(ch, 1)
                ch["fold_for"] = {}
                ch["napply"] = 0
                ch["q"] = None
                ch["mbc"] = {}
              for t in range(TB):
                c = t // CH
                slot = t % RING
                nslot = (t + 1) % RING
                for key, ch in chains.items():
                    n = int(ch["N_t"][t])
                    if (t % CH == 0 and c + 1 < nchunks
                            and ch["echunks"][c + 1] is None):
                        load_chunk(ch, c + 1)

                    q = ch["psum"].tile([104, NMAX], mybir.dt.float32,
                                        tag=f"q{key}")
                    nc.tensor.matmul(
                        q[:, 0:n], ch["wt"][:],
                        ch["ring"][:, slot * NMAX:slot * NMAX + n],
                        start=True, stop=True)
                    ch["q"] = q
                    if t in ch["fold_for"]:
                        e_ap = ch["fold_for"].pop(t)[:, 0:n]
                    else:
                        e_ap = ch["echunks"][c][:, ch["off"][t]
                                                - ch["off"][c * CH]:
                                                ch["off"][t]
                                                - ch["off"][c * CH] + n]
                    out_ap = ch["ring"][:, nslot * NMAX:nslot * NMAX + n]
                    if emul == "stt":
                        ch["ve"].scalar_tensor_tensor(
                            out_ap, q[:, 0:n], 1.0, e_ap,
                            mybir.AluOpType.mult, mybir.AluOpType.mult)
                    elif emul == "tm_vec":
                        nc.vector.tensor_mul(out_ap, q[:, 0:n], e_ap)
                    elif emul == "tm_gps":
                        nc.gpsimd.tensor_mul(out_ap, q[:, 0:n], e_ap)
                    elif emul == "tm_mix":
                        (nc.vector if key == "F" else nc.gpsimd).tensor_mul(
                            out_ap, q[:, 0:n], e_ap)
                    else:  # diagnostic: plain copy, no E operand (WRONG result)
                        ch["ve"].tensor_scalar_mul(out_ap, q[:, 0:n], 1.0)

                # ---- PE p-state warming: tiny-contract filler matmuls ----
                # 4 contract rows (slice of the resident wall tile) keep the
                # PE pipeline busy through the inter-tick stall at ~25ns per
                # filler, so the next chain matmul dispatches back-to-back at
                # mid/full p-state instead of cold (0.65GHz).
                if warm:
                    # disjoint column slices of one tile: no filler-to-filler
                    # deps, so Tile inserts no semaphores between them
                    qd = dummypool.tile([104, warm * warm_w],
                                        mybir.dt.float32, tag="qd")
                    for _w in range(warm):
                        nc.tensor.matmul(
                            qd[:, _w * warm_w:(_w + 1) * warm_w],
                            wallF_t[0:4, :], wallF_t[0:4, 0:warm_w],
                            start=True, stop=True)

                if do_renorm and t in rcp_at:
                    ta = rcp_at[t]
                    for key, ch in chains.items():
                        na = int(ch["N_t"][ta])
                        a_i = ch["napply"]
                        nc.vector.reciprocal(
                            out=ch["mring"][:, a_i * NMAX:a_i * NMAX + na],
                            in_=ch["q"][96:100, 0:na])
                        ch["napply"] += 1
                if do_renorm and t in mbc_at:
                    ta = mbc_at[t]
                    for key, ch in chains.items():
                        na = int(ch["N_t"][ta])
                        a_i = ch["napply"] - 1
                        mslice = ch["mring"][:, a_i * NMAX:a_i * NMAX + na]
                        mbc = mbcpool.tile([104, NMAX], mybir.dt.float32,
                                           tag="mbc")
                        nc.tensor.matmul(mbc[:, 0:na], wones_t[:], mslice,
                                         start=True, stop=True)
                        ch["mbc"][ta] = mbc
                if do_renorm and t in mul_at:
                    ta = mul_at[t]
                    for key, ch in chains.items():
                        na = int(ch["N_t"][ta])
                        mbc = ch["mbc"].pop(ta)
                        ef = efoldpool.tile([104, NMAX], mybir.dt.bfloat16,
                                            tag=f"ef{key}")
                        ca = ta // CH
                        if ch["echunks"][ca] is None:
                            load_chunk(ch, ca)
                        eslice = ch["echunks"][ca][:, ch["off"][ta]
                                                   - ch["off"][ca * CH]:
                                                   ch["off"][ta]
                                                   - ch["off"][ca * CH] + na]
                        nc.vector.tensor_mul(ef[:, 0:na], eslice,
                                             mbc[:, 0:na])
                        ch["fold_for"][ta] = ef

                # ---- state dump (every DUMPG ring slots, by tau = t+1) ----
                tau = t + 1
                if do_dump and (tau % DUMPG == DUMPG - 1 or t == TB - 1):
                    k = tau // DUMPG
                    s0 = (k * DUMPG) % RING
                    for key, ch in chains.items():
                        nc.sync.dma_start(
                            out=ch["dump"][:, k * DUMPG * NMAX:
                                           (k + 1) * DUMPG * NMAX],
                            in_=ch["ring"][:, s0 * NMAX:(s0 + DUMPG) * NMAX])

            if repeat == 1:
                body()
            else:
                with tc.For_i(0, repeat, 1) as _i:
                    body(_i)
            if NAPPLY > 0:
                nc.sync.dma_start(out=mdF[:], in_=mringF[:])
                nc.sync.dma_start(out=mdB[:], in_=mringB[:])
    nc.compile()
    return nc


# ----------------------------------------------------------------------------
# host assembly
# ----------------------------------------------------------------------------
def assemble_fwd(results, sched, mus, lengths, Mstop):
    """results: list of per-core dicts with dumpF/dumpB/mdF/mdB."""
    applies, order = sched["applies"], sched["order"]
    lengths = np.asarray(lengths).astype(np.int64)
    fwd = np.zeros(B, dtype=np.float64)
    ap_arr = np.asarray(applies)
    Mstop64 = Mstop.astype(np.float64)
    for m in range(NCORES):
        shard = order[m::NCORES]
        lens_s = lengths[shard]
        m_s = -(-lens_s // 2)
        k_s = lens_s // 2
        dF = results[m]["dumpF"].astype(np.float32)
        dB = results[m]["dumpB"].astype(np.float32)
        mu_cum = np.cumsum(mus[m], axis=1)                # [256, T]

        def logm_cum(md):
            md = md.astype(np.float64)
            out = np.zeros((len(applies) + 1, 4, NMAX))
            for i, t0 in enumerate(applies):
                nn = None
                blk = np.log(np.maximum(
                    md[:, i * NMAX:(i + 1) * NMAX], 1e-300))
                out[i + 1] = out[i] + blk
            return out

        lmF = logm_cum(results[m]["mdF"])
        lmB = logm_cum(results[m]["mdB"])
        for s in range(SEQ_PER_CORE):
            g, nn = s % G, s // G
            L = int(lens_s[s])
            mm, kk = int(m_s[s]), int(k_s[s])
            rows = ROWMAP[g][:K - 2]                      # 23? no: 25 states
            rows = ROWMAP[g]                              # all 25 (K) states
            a = dF[rows, mm * NMAX + nn].astype(np.float64)
            if kk == 0:
                bvec = Mstop64
            else:
                bvec = dB[rows, kk * NMAX + nn].astype(np.float64)
            # scale corrections: folds at step ta affect states slot >= ta+1
            cF = int(np.searchsorted(ap_arr, mm - 1, "right")) if len(
                ap_arr) else 0
            cB = int(np.searchsorted(ap_arr, kk - 1, "right")) if len(
                ap_arr) else 0
            scale = lmF[cF][g, nn] + (lmB[cB][g, nn] if kk > 0 else 0.0)
            z = float(np.dot(a, bvec))
            fwd[shard[s]] = (np.log(max(z, 1e-300)) - scale
                             + mu_cum[s, L - 1])
    return fwd


def gold_scores(feats, tags, lengths, transitions):
    f = feats.astype(np.float64)
    tr = transitions.astype(np.float64)
    tags = np.asarray(tags).astype(np.int64)
    lengths = np.asarray(lengths).astype(np.int64)
    mask = np.arange(T)[None, :] < lengths[:, None]
    tags_ext = np.concatenate(
        [np.full((B, 1), START, dtype=np.int64), tags], axis=1)
    trans_sc = tr[tags_ext[:, 1:], tags_ext[:, :-1]]
    emit_sc = np.take_along_axis(f, tags[..., None], axis=-1)[..., 0]
    last_tag = np.take_along_axis(tags, (lengths - 1)[:, None], axis=1)[:, 0]
    return ((trans_sc + emit_sc) * mask).sum(1) + tr[STOP, last_tag]


# ----------------------------------------------------------------------------
# entry point
# ----------------------------------------------------------------------------
def make_executor(nc):
    """Build a reusable sharded PJRT callable for `nc` (8-core SPMD)."""
    import jax
    from jax.sharding import Mesh, PartitionSpec
    from jax.experimental.shard_map import shard_map
    from concourse import mybir
    from concourse.bass2jax import (_bass_exec_p, install_neuronx_cc_hook,
                                    partition_id_tensor)

    install_neuronx_cc_hook()
    in_names, out_names, out_avals, zero_outs = [], [], [], []
    partition_name = (nc.partition_id_tensor.name
                      if nc.partition_id_tensor else None)
    for alloc in nc.m.functions[0].allocations:
        if not isinstance(alloc, mybir.MemoryLocationSet):
            continue
        name = alloc.memorylocations[0].name
        if alloc.kind == "ExternalInput":
            if name != partition_name:
                in_names.append(name)
        elif alloc.kind == "ExternalOutput":
            out_names.append(name)
            shape = tuple(alloc.tensor_shape)
            dtype = mybir.dt.np(alloc.dtype)
            out_avals.append(jax.core.ShapedArray(shape, dtype))
            zero_outs.append(np.zeros(shape, dtype))
    n_params = len(in_names)
    n_outs = len(out_avals)
    all_in_names = list(in_names) + list(out_names)
    if partition_name is not None:
        all_in_names.append(partition_name)
    donate = tuple(range(n_params, n_params + n_outs))

    def _body(*args):
        operands = list(args)
        if partition_name is not None:
            operands.append(partition_id_tensor())
        return tuple(_bass_exec_p.bind(
            *operands,
            out_avals=tuple(out_avals),
            in_names=tuple(all_in_names),
            out_names=tuple(out_names),
            lowering_input_output_aliases=(),
            sim_require_finite=False,
            sim_require_nnan=False,
            nc=nc,
        ))

    devices = [d for d in jax.devices() if d.platform != "cpu"]
    if len(devices) < NCORES:
        devices = jax.devices("axon")
    devices = devices[:NCORES]
    assert len(devices) == NCORES, f"need {NCORES} neuron cores, {devices=}"
    mesh = Mesh(np.asarray(devices), ("core",))
    in_specs = (PartitionSpec("core"),) * (n_params + n_outs)
    out_specs = (PartitionSpec("core"),) * n_outs
    sharded = jax.jit(
        shard_map(_body, mesh=mesh, in_specs=in_specs, out_specs=out_specs,
                  check_rep=False),
        donate_argnums=donate, keep_unused=True)

    def prep_inputs(in_maps):
        concat = [np.concatenate([np.asarray(in_maps[c][nm])
                                  for c in range(NCORES)], axis=0)
                  for nm in in_names]
        sh = jax.sharding.NamedSharding(mesh, PartitionSpec("core"))
        return [jax.device_put(a, sh) for a in concat]

    def prep_zeros():
        sh = jax.sharding.NamedSharding(mesh, PartitionSpec("core"))
        return [jax.device_put(
            np.zeros((NCORES * z.shape[0], *z.shape[1:]), z.dtype), sh)
            for z in zero_outs]

    def run(dev_inputs, dev_zeros):
        outs = sharded(*dev_inputs, *dev_zeros)
        jax.block_until_ready(outs)
        return outs

    def split(outs):
        res = [dict() for _ in range(NCORES)]
        for i, nm in enumerate(out_names):
            arr = np.asarray(outs[i])
            per = arr.shape[0] // NCORES
            for c in range(NCORES):
                res[c][nm] = arr[c * per:(c + 1) * per]
        return res

    return dict(prep_inputs=prep_inputs, prep_zeros=prep_zeros, run=run,
                split=split)


def prepare_all(feats, lengths, transitions, renorm_w=None):
    """Schedule + per-core input maps.  Returns (sched, in_maps, mus, Mstop).
    renorm_w=None (default): host-side normalization folded into E, no
    device renorm ops.  renorm_w=<int>: device renorm every <int> steps."""
    sched = make_schedule(lengths, renorm_w=renorm_w)
    order = sched["order"]
    lengths = np.asarray(lengths).astype(np.int64)
    tr32 = np.asarray(transitions, dtype=np.float32)
    wallF, wallB, Mstop = build_walls(tr32)
    c0 = None
    if not renorm_w:
        colsum = np.exp(tr32.astype(np.float64)).sum(0)        # [K]
        c0 = float(np.log(colsum.mean()) - np.log(K))
    p0 = build_p0()
    wones = build_wones()
    in_maps, mus = [], []
    feats = np.asarray(feats, dtype=np.float32)
    for m in range(NCORES):
        shard = order[m::NCORES]
        core_in, mu = build_core_inputs(feats[shard], lengths[shard], sched,
                                        Mstop, c0=c0)
        in_maps.append({"efF": core_in["efF"], "efB": core_in["efB"],
                        "p0F": p0, "p0B": core_in["p0B"],
                        "wallF": wallF, "wallB": wallB, "wones": wones})
        mus.append(mu)
    return sched, in_maps, mus, Mstop


def kernel(feats, tags, lengths, transitions):
    feats = np.asarray(feats, dtype=np.float32)
    lengths_np = np.asarray(lengths)
    sched, in_maps, mus, Mstop = prepare_all(feats, lengths_np, transitions)
    nc = build_nc(sched)
    ex = make_executor(nc)
    dev_in = ex["prep_inputs"](in_maps)
    results = ex["split"](ex["run"](dev_in, ex["prep_zeros"]()))
    fwd = assemble_fwd(results, sched, mus, lengths_np, Mstop)
    gold = gold_scores(feats, tags, lengths_np,
                       np.asarray(transitions, dtype=np.float32))
    return np.float32((fwd - gold).mean())


# revision 37
# speedup vs baseline: 1.0026x; 1.0026x over previous
"""Trainium2 Bass kernel for batched CRF negative log-likelihood.

Meet-in-the-middle forward algorithm (device), probability space:
  forward chain:  a_{t+1} = (Wf @ a_t) * EF_t,  a_0 = onehot(START)
  backward chain: v_{k+1} = (Wb @ v_k) * EB_k,  v_0 = Wstop * E_{L-1}
     where Wb = W^T per 25-state group; EB_k = E_{L-2-k}, except at a
     sequence's final backward step where EB = 1 (so v_K = beta_m pure).
  Z = sum_j a_m[j] * beta_m[j],  m = ceil(L/2), K = floor(L/2).
Both chains are independent 256-tick scans (vs 513 for pure forward),
halving the sequential-latency-bound wall time.

Layout: 4 groups of 25 states on partitions (104 rows incl 4 renorm
rows); 64 columns per group = 256 seqs/core, sorted by length (desc)
and dealt round-robin over 8 cores.  Columns deactivate as chains end
(compile-time schedule).  Every W steps the state is rescaled by
m = 1/r (r = stop-projection for F, group-sum for B) folded into the E
tile; m is dumped so the host can undo it exactly.  The full state ring
is dumped to DRAM every DUMPG ticks; the host reads a_m / beta_m at each
sequence's own meet tick, computes log(a_m . beta_m) + scale corrections
+ cumulative feat-max, subtracts the gold path score, and averages.
"""

import sys

sys.path.insert(0, "/opt/trn_rl_repo")

import numpy as np
import ml_dtypes

bf16 = ml_dtypes.bfloat16

# ---- problem constants (hardcoded per contest rules) ----
B, T, OUT = 2048, 512, 23
K = OUT + 2
START, STOP = OUT, OUT + 1
NEG = -10000.0

NCORES = 8
G = 4            # state groups on partitions (4 x 25 = 100 state rows)
NMAX = 64        # max columns = (2048/8)/G
RING = 32        # p ring depth (steps)
W = 16           # renormalization period (steps)
LAG = 4          # staleness of r used for renormalization (= prep lead time)
CH = 32          # E-chunk size in steps
DUMPG = 16       # state-dump group size (ring slots per dump DMA)
SEQ_PER_CORE = B // NCORES


# ----------------------------------------------------------------------------
# schedule (compile-time, from lengths)
# ----------------------------------------------------------------------------
def make_schedule(lengths, renorm_w=W):
    lengths = np.asarray(lengths).astype(np.int64)
    order = np.argsort(-lengths, kind="stable")
    mhalf = -(-lengths // 2)                  # m_s = ceil(L/2): forward ticks
    khalf = lengths // 2                      # K_s = floor(L/2): backward ticks
    TB = int(max(mhalf.max(), khalf.max()))   # device steps 0..TB-1

    def widths(active_ticks):
        # N_t = cols processed at step t; col for seq alive while t < ticks
        Ag = np.array([(active_ticks > t).sum() for t in range(TB)],
                      dtype=np.int64)
        Acore = -(-Ag // NCORES)
        N_t = np.maximum(1, -(-Acore // G)).astype(int)
        off = np.zeros(TB + 1, dtype=np.int64)
        for t in range(TB):
            off[t + 1] = off[t] + N_t[t]
        return N_t, off, int(off[TB])

    NF_t, offF, ECF = widths(mhalf)
    NB_t, offB, ECB = widths(khalf)
    # device renorm steps; empty when the host pre-folds scales into E
    applies = list(range(renorm_w, TB, renorm_w)) if renorm_w else []
    return dict(order=order, mhalf=mhalf, khalf=khalf, TB=TB,
                NF_t=NF_t, offF=offF, ECF=ECF,
                NB_t=NB_t, offB=offB, ECB=ECB, applies=applies)


# ----------------------------------------------------------------------------
# host-side input preparation (per core)
# ----------------------------------------------------------------------------
def pos(g, j):
    """Partition of state j of group g.  r-rows live at 96..99 (32-aligned
    for the dump DMA / rcp reads); group 3's states fill 75..95 + 100..103."""
    if g < 3:
        return 25 * g + j
    return 75 + j if j < 21 else 100 + (j - 21)


def rpos(g):
    return 96 + g


ROWMAP = np.array([[pos(g, j) for j in range(K)] for g in range(G)])


def build_walls(transitions):
    M = np.exp(transitions.astype(np.float64)).astype(np.float32)      # [K, K]
    Mstop = np.exp(transitions[STOP].astype(np.float64)).astype(np.float32)
    Wf = np.zeros((104, 104), dtype=np.float32)   # [out_row, in_row]
    Wb = np.zeros((104, 104), dtype=np.float32)
    for g in range(G):
        for jo in range(K):
            for ji in range(K):
                Wf[pos(g, jo), pos(g, ji)] = M[jo, ji]
                Wb[pos(g, jo), pos(g, ji)] = M[ji, jo]   # transposed block
        for ji in range(K):
            Wf[rpos(g), pos(g, ji)] = Mstop[ji]          # stop projection
            Wb[rpos(g), pos(g, ji)] = 1.0                # group mass
    lhsTf = np.ascontiguousarray(Wf.T).astype(bf16)      # [in(contract), out]
    lhsTb = np.ascontiguousarray(Wb.T).astype(bf16)
    return lhsTf, lhsTb, Mstop


def build_p0():
    p0 = np.zeros((104, NMAX), dtype=np.float32)
    for g in range(G):
        p0[pos(g, START), :] = 1.0
    return p0.astype(bf16)


def build_wones():
    """lhsT for the m-broadcast matmul: out[:, c] = ones_block @ m[:, c].
    All of group g's state rows and its r-row get m[g]."""
    w = np.zeros((4, 104), dtype=np.float32)
    for g in range(G):
        for j in range(K):
            w[g, pos(g, j)] = 1.0
        w[g, rpos(g)] = 1.0
    return w


def build_core_inputs(feats_shard, lens_s, sched, Mstop, c0=None):
    """feats_shard: [256, T, K] f32 for this core (order-sorted slice).
    Returns dict of device input arrays + mu [256, T]."""
    m_s = -(-lens_s // 2)
    k_s = lens_s // 2
    NF_t, offF, ECF = sched["NF_t"], sched["offF"], sched["ECF"]
    NB_t, offB, ECB = sched["NB_t"], sched["offB"], sched["ECB"]
    TB = sched["TB"]

    mu = feats_shard.max(-1)                                   # [256, T]
    Eraw = np.exp(feats_shard - mu[..., None])                 # [256, T, K]
    if c0 is not None:
        # host-side renorm: fold the estimated per-tick log-growth into E
        # (exactly like mu); undone via mu_cum in the assembly.
        inc = np.clip(np.log(Eraw.sum(-1)) + c0, -30.0, 30.0)  # [256, T]
        Eraw = Eraw * np.exp(-inc)[..., None]
        mu = mu + inc
    E = Eraw.astype(bf16)                                      # [256, T, K]
    # seq s = n*G + g  ->  row pos(g, j), col n
    Er = E.reshape(NMAX, G, T, K).transpose(1, 3, 2, 0)        # [G, K, T, NMAX]

    efF = np.ones((104, ECF), dtype=bf16)
    for t in range(TB):
        n = NF_t[t]
        for g in range(G):
            efF[ROWMAP[g], offF[t]:offF[t] + n] = Er[g, :, t, :n]

    # backward: step t uses E_{L-2-t} for each seq, ones at final step t=K-1
    efB = np.ones((104, ECB), dtype=bf16)
    tidx = np.asarray(lens_s) - 2 - np.arange(TB)[:, None]     # [TB, 256]
    for t in range(TB):
        n = NB_t[t]
        for g in range(G):
            for nn in range(n):
                s = nn * G + g
                if t >= k_s[s]:          # dead (rounding slack): leave ones
                    continue
                if t == k_s[s] - 1:      # final backward step: no emission
                    continue
                efB[ROWMAP[g], offB[t] + nn] = Er[g, :, tidx[t, s], nn]

    # v_0 = Wstop * E_{L-1} per seq
    p0B = np.ones((104, NMAX), dtype=np.float32)
    EL = E[np.arange(SEQ_PER_CORE), np.asarray(lens_s) - 1].astype(np.float32)
    for g in range(G):
        for nn in range(NMAX):
            s = nn * G + g
            p0B[ROWMAP[g], nn] = Mstop * EL[s]
    return dict(efF=efF, efB=efB, p0B=p0B.astype(bf16)), mu


# ----------------------------------------------------------------------------
# device kernel builder
# ----------------------------------------------------------------------------
def build_nc(sched, repeat=1, eng_f="vector", eng_b="vector", warm=0,
             warm_w=48, do_dump=True, do_renorm=True, emul="stt", pbufs=2):
    import concourse.bass as bass
    import concourse.tile as tile
    from concourse import bacc, mybir

    TB, applies = sched["TB"], sched["applies"]
    NAPPLY = len(applies)
    NDUMP = -(-(TB + 1) // DUMPG)

    nc = bacc.Bacc("TRN2", target_bir_lowering=False, debug=False,
                   num_devices=NCORES)
    efF = nc.dram_tensor("efF", [104, sched["ECF"]], mybir.dt.bfloat16,
                         kind="ExternalInput").ap()
    efB = nc.dram_tensor("efB", [104, sched["ECB"]], mybir.dt.bfloat16,
                         kind="ExternalInput").ap()
    p0F = nc.dram_tensor("p0F", [104, NMAX], mybir.dt.bfloat16,
                         kind="ExternalInput").ap()
    p0B = nc.dram_tensor("p0B", [104, NMAX], mybir.dt.bfloat16,
                         kind="ExternalInput").ap()
    wallF = nc.dram_tensor("wallF", [104, 104], mybir.dt.bfloat16,
                           kind="ExternalInput").ap()
    wallB = nc.dram_tensor("wallB", [104, 104], mybir.dt.bfloat16,
                           kind="ExternalInput").ap()
    wones = nc.dram_tensor("wones", [4, 104], mybir.dt.float32,
                           kind="ExternalInput").ap()
    dumpF = nc.dram_tensor("dumpF", [104, NDUMP * DUMPG * NMAX],
                           mybir.dt.bfloat16, kind="ExternalOutput").ap()
    dumpB = nc.dram_tensor("dumpB", [104, NDUMP * DUMPG * NMAX],
                           mybir.dt.bfloat16, kind="ExternalOutput").ap()
    mdF = nc.dram_tensor("mdF", [4, max(1, NAPPLY) * NMAX], mybir.dt.float32,
                         kind="ExternalOutput").ap()
    mdB = nc.dram_tensor("mdB", [4, max(1, NAPPLY) * NMAX], mybir.dt.float32,
                         kind="ExternalOutput").ap()

    with tile.TileContext(nc) as tc:
        from contextlib import ExitStack
        with ExitStack() as ctx:
            singles = ctx.enter_context(tc.tile_pool(name="singles", bufs=1))
            epoolF = ctx.enter_context(tc.tile_pool(name="epoolF", bufs=3))
            epoolB = ctx.enter_context(tc.tile_pool(name="epoolB", bufs=3))
            psumF = ctx.enter_context(
                tc.tile_pool(name="psumF", bufs=pbufs, space="PSUM"))
            psumB = ctx.enter_context(
                tc.tile_pool(name="psumB", bufs=pbufs, space="PSUM"))
            mbcpool = ctx.enter_context(
                tc.tile_pool(name="mbcpool", bufs=2, space="PSUM"))
            efoldpool = ctx.enter_context(tc.tile_pool(name="efoldpool",
                                                       bufs=4))
            dummypool = (ctx.enter_context(
                tc.tile_pool(name="dummypool", bufs=2, space="PSUM"))
                if warm else None)

            wallF_t = singles.tile([104, 104], mybir.dt.bfloat16)
            nc.sync.dma_start(out=wallF_t[:], in_=wallF[:])
            wallB_t = singles.tile([104, 104], mybir.dt.bfloat16)
            nc.sync.dma_start(out=wallB_t[:], in_=wallB[:])
            wones_t = singles.tile([4, 104], mybir.dt.float32)
            nc.sync.dma_start(out=wones_t[:], in_=wones[:])
            ringF = singles.tile([104, RING * NMAX], mybir.dt.bfloat16)
            nc.vector.memset(ringF[:, NMAX:], 0.0)
            nc.sync.dma_start(out=ringF[:, 0:NMAX], in_=p0F[:])
            ringB = singles.tile([104, RING * NMAX], mybir.dt.bfloat16)
            nc.vector.memset(ringB[:, NMAX:], 0.0)
            nc.sync.dma_start(out=ringB[:, 0:NMAX], in_=p0B[:])
            mringF = singles.tile([4, max(1, NAPPLY) * NMAX], mybir.dt.float32)
            nc.vector.memset(mringF[:], 1.0)
            mringB = singles.tile([4, max(1, NAPPLY) * NMAX], mybir.dt.float32)
            nc.vector.memset(mringB[:], 1.0)

            nchunks = -(-TB // CH)

            engs = {"vector": nc.vector, "gpsimd": nc.gpsimd}
            chains = {}
            for key, epool, psum, mring, ring, wt, ef, dump, N_t, off, ve in (
                ("F", epoolF, psumF, mringF, ringF, wallF_t, efF, dumpF,
                 sched["NF_t"], sched["offF"], engs[eng_f]),
                ("B", epoolB, psumB, mringB, ringB, wallB_t, efB, dumpB,
                 sched["NB_t"], sched["offB"], engs[eng_b])):
                chains[key] = dict(epool=epool, psum=psum, mring=mring,
                                   ring=ring, wt=wt, ef=ef, dump=dump,
                                   N_t=N_t, off=off, ve=ve,
                                   chunk_w=[int(off[min((c + 1) * CH, TB)]
                                                - off[c * CH])
                                            for c in range(nchunks)],
                                   echunks=[None] * nchunks)
            for ch in chains.values():
                ch["maxw"] = max(ch["chunk_w"])

            def load_chunk(ch, c):
                wdt = ch["chunk_w"][c]
                et = ch["epool"].tile([104, ch["maxw"]], mybir.dt.bfloat16,
                                      tag="E")
                a = int(ch["off"][c * CH])
                nc.sync.dma_start(out=et[:, 0:wdt], in_=ch["ef"][:, a:a + wdt])
                ch["echunks"][c] = et

            # renorm pipeline, spread so no engine stalls another:
            #   tick ta-LAG:   reciprocal of r rows (DVE, after stts)
            #   tick ta-LAG+1: broadcast matmul (PE, after chain matmuls)
            #   tick ta-LAG+2: fold multiply into E copy (DVE)
            #   tick ta:       stt consumes folded E
            rcp_at = {t - LAG: t for t in applies}
            mbc_at = {t - LAG + 1: t for t in applies}
            mul_at = {t - LAG + 2: t for t in applies}

            def body(_i=None):
              if _i is not None:
                nc.sync.dma_start(out=ringF[:, 0:NMAX], in_=p0F[:])
                nc.sync.dma_start(out=ringB[:, 0:NMAX], in_=p0B[:])
              for ch in chains.values():
                for c_ in range(nchunks):
                    ch["echunks"][c_] = None
                load_chunk(ch, 0)
                if nchunks > 1:
                    load_chunk(ch, 1)
                ch["fold_for"] = {}
                ch["napply"] = 0
                ch["q"] = None
                ch["mbc"] = {}
              for t in range(TB):
                c = t // CH
                slot = t % RING
                nslot = (t + 1) % RING
                for key, ch in chains.items():
                    n = int(ch["N_t"][t])
                    if (t % CH == 0 and c + 1 < nchunks
                            and ch["echunks"][c + 1] is None):
                        load_chunk(ch, c + 1)

                    q = ch["psum"].tile([104, NMAX], mybir.dt.float32,
                                        tag=f"q{key}")
                    nc.tensor.matmul(
                        q[:, 0:n], ch["wt"][:],
                        ch["ring"][:, slot * NMAX:slot * NMAX + n],
                        start=True, stop=True)
                    ch["q"] = q
                    if t in ch["fold_for"]:
                        e_ap = ch["fold_for"].pop(t)[:, 0:n]
                    else:
                        e_ap = ch["echunks"][c][:, ch["off"][t]
                                                - ch["off"][c * CH]:
                                                ch["off"][t]
                                                - ch["off"][c * CH] + n]
                    out_ap = ch["ring"][:, nslot * NMAX:nslot * NMAX + n]
                    if emul == "stt":
                        ch["ve"].scalar_tensor_tensor(
                            out_ap, q[:, 0:n], 1.0, e_ap,
                            mybir.AluOpType.mult, mybir.AluOpType.mult)
                    elif emul == "tm_vec":
                        nc.vector.tensor_mul(out_ap, q[:, 0:n], e_ap)
                    elif emul == "tm_gps":
                        nc.gpsimd.tensor_mul(out_ap, q[:, 0:n], e_ap)
                    elif emul == "tm_mix":
                        (nc.vector if key == "F" else nc.gpsimd).tensor_mul(
                            out_ap, q[:, 0:n], e_ap)
                    else:  # diagnostic: plain copy, no E operand (WRONG result)
                        ch["ve"].tensor_scalar_mul(out_ap, q[:, 0:n], 1.0)

                # ---- PE p-state warming: tiny-contract filler matmuls ----
                # 4 contract rows (slice of the resident wall tile) keep the
                # PE pipeline busy through the inter-tick stall at ~25ns per
                # filler, so the next chain matmul dispatches back-to-back at
                # mid/full p-state instead of cold (0.65GHz).
                if warm:
                    # disjoint column slices of one tile: no filler-to-filler
                    # deps, so Tile inserts no semaphores between them
                    qd = dummypool.tile([104, warm * warm_w],
                                        mybir.dt.float32, tag="qd")
                    for _w in range(warm):
                        nc.tensor.matmul(
                            qd[:, _w * warm_w:(_w + 1) * warm_w],
                            wallF_t[0:4, :], wallF_t[0:4, 0:warm_w],
                            start=True, stop=True)

                if do_renorm and t in rcp_at:
                    ta = rcp_at[t]
                    for key, ch in chains.items():
                        na = int(ch["N_t"][ta])
                        a_i = ch["napply"]
                        nc.vector.reciprocal(
                            out=ch["mring"][:, a_i * NMAX:a_i * NMAX + na],
                            in_=ch["q"][96:100, 0:na])
                        ch["napply"] += 1
                if do_renorm and t in mbc_at:
                    ta = mbc_at[t]
                    for key, ch in chains.items():
                        na = int(ch["N_t"][ta])
                        a_i = ch["napply"] - 1
                        mslice = ch["mring"][:, a_i * NMAX:a_i * NMAX + na]
                        mbc = mbcpool.tile([104, NMAX], mybir.dt.float32,
                                           tag="mbc")
                        nc.tensor.matmul(mbc[:, 0:na], wones_t[:], mslice,
                                         start=True, stop=True)
                        ch["mbc"][ta] = mbc
                if do_renorm and t in mul_at:
                    ta = mul_at[t]
                    for key, ch in chains.items():
                        na = int(ch["N_t"][ta])
                        mbc = ch["mbc"].pop(ta)
                        ef = efoldpool.tile([104, NMAX], mybir.dt.bfloat16,
                                            tag=f"ef{key}")
                        ca = ta // CH
                        if ch["echunks"][ca] is None:
                            load_chunk(ch, ca)
                        eslice = ch["echunks"][ca][:, ch["off"][ta]
                                                   - ch["off"][ca * CH]:
                                                   ch["off"][ta]
                                                   - ch["off"][ca * CH] + na]
                        nc.vector.tensor_mul(ef[:, 0:na], eslice,
                                             mbc[:, 0:na])
                        ch["fold_for"][ta] = ef

                # ---- state dump (every DUMPG ring slots, by tau = t+1) ----
                tau = t + 1
                if do_dump and (tau % DUMPG == DUMPG - 1 or t == TB - 1):
                    k = tau // DUMPG
                    s0 = (k * DUMPG) % RING
                    for key, ch in chains.items():
                        nc.sync.dma_start(
                            out=ch["dump"][:, k * DUMPG * NMAX:
                                           (k + 1) * DUMPG * NMAX],
                            in_=ch["ring"][:, s0 * NMAX:(s0 + DUMPG) * NMAX])

            if repeat == 1:
                body()
            else:
                with tc.For_i(0, repeat, 1) as _i:
                    body(_i)
            if NAPPLY > 0:
                nc.sync.dma_start(out=mdF[:], in_=mringF[:])
                nc.sync.dma_start(out=mdB[:], in_=mringB[:])
    nc.compile()
    return nc


# ----------------------------------------------------------------------------
# host assembly
# ----------------------------------------------------------------------------
def assemble_fwd(results, sched, mus, lengths, Mstop):
    """results: list of per-core dicts with dumpF/dumpB/mdF/mdB."""
    applies, order = sched["applies"], sched["order"]
    lengths = np.asarray(lengths).astype(np.int64)
    fwd = np.zeros(B, dtype=np.float64)
    ap_arr = np.asarray(applies)
    Mstop64 = Mstop.astype(np.float64)
    for m in range(NCORES):
        shard = order[m::NCORES]
        lens_s = lengths[shard]
        m_s = -(-lens_s // 2)
        k_s = lens_s // 2
        dF = results[m]["dumpF"].astype(np.float32)
        dB = results[m]["dumpB"].astype(np.float32)
        mu_cum = np.cumsum(mus[m], axis=1)                # [256, T]

        def logm_cum(md):
            md = md.astype(np.float64)
            out = np.zeros((len(applies) + 1, 4, NMAX))
            for i, t0 in enumerate(applies):
                nn = None
                blk = np.log(np.maximum(
                    md[:, i * NMAX:(i + 1) * NMAX], 1e-300))
                out[i + 1] = out[i] + blk
            return out

        lmF = logm_cum(results[m]["mdF"])
        lmB = logm_cum(results[m]["mdB"])
        for s in range(SEQ_PER_CORE):
            g, nn = s % G, s // G
            L = int(lens_s[s])
            mm, kk = int(m_s[s]), int(k_s[s])
            rows = ROWMAP[g][:K - 2]                      # 23? no: 25 states
            rows = ROWMAP[g]                              # all 25 (K) states
            a = dF[rows, mm * NMAX + nn].astype(np.float64)
            if kk == 0:
                bvec = Mstop64
            else:
                bvec = dB[rows, kk * NMAX + nn].astype(np.float64)
            # scale corrections: folds at step ta affect states slot >= ta+1
            cF = int(np.searchsorted(ap_arr, mm - 1, "right")) if len(
                ap_arr) else 0
            cB = int(np.searchsorted(ap_arr, kk - 1, "right")) if len(
                ap_arr) else 0
            scale = lmF[cF][g, nn] + (lmB[cB][g, nn] if kk > 0 else 0.0)
            z = float(np.dot(a, bvec))
            fwd[shard[s]] = (np.log(max(z, 1e-300)) - scale
                             + mu_cum[s, L - 1])
    return fwd


def gold_scores(feats, tags, lengths, transitions):
    f = feats.astype(np.float64)
    tr = transitions.astype(np.float64)
    tags = np.asarray(tags).astype(np.int64)
    lengths = np.asarray(lengths).astype(np.int64)
    mask = np.arange(T)[None, :] < lengths[:, None]
    tags_ext = np.concatenate(
        [np.full((B, 1), START, dtype=np.int64), tags], axis=1)
    trans_sc = tr[tags_ext[:, 1:], tags_ext[:, :-1]]
    emit_sc = np.take_along_axis(f, tags[..., None], axis=-1)[..., 0]
    last_tag = np.take_along_axis(tags, (lengths - 1)[:, None], axis=1)[:, 0]
    return ((trans_sc + emit_sc) * mask).sum(1) + tr[STOP, last_tag]


# ----------------------------------------------------------------------------
# entry point
# ----------------------------------------------------------------------------
def make_executor(nc):
    """Build a reusable sharded PJRT callable for `nc` (8-core SPMD)."""
    import jax
    from jax.sharding import Mesh, PartitionSpec
    from jax.experimental.shard_map import shard_map
    from concourse import mybir
    from concourse.bass2jax import (_bass_exec_p, install_neuronx_cc_hook,
                                    partition_id_tensor)

    install_neuronx_cc_hook()
    in_names, out_names, out_avals, zero_outs = [], [], [], []
    partition_name = (nc.partition_id_tensor.name
                      if nc.partition_id_tensor else None)
    for alloc in nc.m.functions[0].allocations:
        if not isinstance(alloc, mybir.MemoryLocationSet):
            continue
        name = alloc.memorylocations[0].name
        if alloc.kind == "ExternalInput":
            if name != partition_name:
                in_names.append(name)
        elif alloc.kind == "ExternalOutput":
            out_names.append(name)
            shape = tuple(alloc.tensor_shape)
            dtype = mybir.dt.np(alloc.dtype)
            out_avals.append(jax.core.ShapedArray(shape, dtype))
            zero_outs.append(np.zeros(shape, dtype))
    n_params = len(in_names)
    n_outs = len(out_avals)
    all_in_names = list(in_names) + list(out_names)
    if partition_name is not None:
        all_in_names.append(partition_name)
    donate = tuple(range(n_params, n_params + n_outs))

    def _body(*args):
        operands = list(args)
        if partition_name is not None:
            operands.append(partition_id_tensor())
        return tuple(_bass_exec_p.bind(
            *operands,
            out_avals=tuple(out_avals),
            in_names=tuple(all_in_names),
            out_names=tuple(out_names),
            lowering_input_output_aliases=(),
            sim_require_finite=False,
            sim_require_nnan=False,
            nc=nc,
        ))

    devices = [d for d in jax.devices() if d.platform != "cpu"]
    if len(devices) < NCORES:
        devices = jax.devices("axon")
    devices = devices[:NCORES]
    assert len(devices) == NCORES, f"need {NCORES} neuron cores, {devices=}"
    mesh = Mesh(np.asarray(devices), ("core",))
    in_specs = (PartitionSpec("core"),) * (n_params + n_outs)
    out_specs = (PartitionSpec("core"),) * n_outs
    sharded = jax.jit(
        shard_map(_body, mesh=mesh, in_specs=in_specs, out_specs=out_specs,
                  check_rep=False),
        donate_argnums=donate, keep_unused=True)

    def prep_inputs(in_maps):
        concat = [np.concatenate([np.asarray(in_maps[c][nm])
                                  for c in range(NCORES)], axis=0)
                  for nm in in_names]
        sh = jax.sharding.NamedSharding(mesh, PartitionSpec("core"))
        return [jax.device_put(a, sh) for a in concat]

    def prep_zeros():
        sh = jax.sharding.NamedSharding(mesh, PartitionSpec("core"))
        return [jax.device_put(
            np.zeros((NCORES * z.shape[0], *z.shape[1:]), z.dtype), sh)
            for z in zero_outs]

    def run(dev_inputs, dev_zeros):
        outs = sharded(*dev_inputs, *dev_zeros)
        jax.block_until_ready(outs)
        return outs

    def split(outs):
        res = [dict() for _ in range(NCORES)]
        for i, nm in enumerate(out_names):
            arr = np.asarray(outs[i])
            per = arr.shape[0] // NCORES
            for c in range(NCORES):
                res[c][nm] = arr[c * per:(c + 1) * per]
        return res

    return dict(prep_inputs=prep_inputs, prep_zeros=prep_zeros, run=run,
                split=split)


def prepare_all(feats, lengths, transitions, renorm_w=None):
    """Schedule + per-core input maps.  Returns (sched, in_maps, mus, Mstop).
    renorm_w=None (default): host-side normalization folded into E, no
    device renorm ops.  renorm_w=<int>: device renorm every <int> steps."""
    sched = make_schedule(lengths, renorm_w=renorm_w)
    order = sched["order"]
    lengths = np.asarray(lengths).astype(np.int64)
    tr32 = np.asarray(transitions, dtype=np.float32)
    wallF, wallB, Mstop = build_walls(tr32)
    c0 = None
    if not renorm_w:
        colsum = np.exp(tr32.astype(np.float64)).sum(0)        # [K]
        c0 = float(np.log(colsum.mean()) - np.log(K))
    p0 = build_p0()
    wones = build_wones()
    in_maps, mus = [], []
    feats = np.asarray(feats, dtype=np.float32)
    for m in range(NCORES):
        shard = order[m::NCORES]
        core_in, mu = build_core_inputs(feats[shard], lengths[shard], sched,
                                        Mstop, c0=c0)
        in_maps.append({"efF": core_in["efF"], "efB": core_in["efB"],
                        "p0F": p0, "p0B": core_in["p0B"],
                        "wallF": wallF, "wallB": wallB, "wones": wones})
        mus.append(mu)
    return sched, in_maps, mus, Mstop


def kernel(feats, tags, lengths, transitions):
    feats = np.asarray(feats, dtype=np.float32)
    lengths_np = np.asarray(lengths)
    sched, in_maps, mus, Mstop = prepare_all(feats, lengths_np, transitions)
    nc = build_nc(sched)
    ex = make_executor(nc)
    dev_in = ex["prep_inputs"](in_maps)
    results = ex["split"](ex["run"](dev_in, ex["prep_zeros"]()))
    fwd = assemble_fwd(results, sched, mus, lengths_np, Mstop)
    gold = gold_scores(feats, tags, lengths_np,
                       np.asarray(transitions, dtype=np.float32))
    return np.float32((fwd - gold).mean())
